# revision 22
# baseline (speedup 1.0000x reference)
"""Coord2HeatmapNet Trainium2 kernel.

out[b,c,j,i] = 10*exp(-(((i+.5)/128 - x)^2 + ((j+.5)/128 - y)^2) / (2*(2/128)^2))

Exploited structure:
  * Separable: each heatmap = fy[j] (x) fx[i] outer product.
  * The gaussian decays fast: a WIN-row window per heatmap carries everything
    above ~2.2e-2 absolute (tolerance is 0.2 abs); the pre-zeroed output
    buffers keep the rest at 0.
  * Derivative_Erf activation = 2/sqrt(pi)*exp(-t^2): one ScalarE op per
    gaussian factor vector.
  * Per-heatmap scalar tables (activation biases, scatter offsets) are tiny
    index arithmetic -> precomputed on host, DMAed in as one [128,_] table.
  * Layout: one heatmap per PARTITION. Partition p of group g holds the whole
    WINx128 window of heatmap k=g*128+p as WIN*128 contiguous floats. The
    outer product is split between DVE (leading rows, one broadcast
    tensor_tensor) and ScalarE (trailing SROWS rows, activation-Copy with
    per-partition scale); the write-out is ONE indirect scatter DMA per group
    (one offset per partition, WIN*512B contiguous per heatmap at its
    data-dependent window position).
  * Each group scatters into its OWN ExternalOutput dram tensor -> no
    write-after-write hazard between groups, so the scatters overlap on the
    16 SDMA engines instead of serializing on completion semaphores.
  * A dummy warmup scatter (to a throwaway output tensor) right after pool
    setup absorbs the ~1us first-indirect-DMA ucode warmup off the critical
    path.

Sharding: pure data parallel, 8 batches per core across 8 NeuronCores.
"""
import sys

for _p in ("/opt/trn_rl_repo", "/root/.axon_site", "/root/.axon_site/_ro/trn_rl_repo",
           "/root/.axon_site/_ro/pypackages"):
    if _p not in sys.path:
        sys.path.append(_p)

import numpy as np

S = 128
NUM_CLASS = 68
B_TOTAL = 64
N_CORES = 8
B_LOC = B_TOTAL // N_CORES            # 8 batches per core
NHM = B_LOC * NUM_CLASS               # 544 heatmaps per core
WIN = 12                              # window rows per heatmap
NG_FULL = NHM // 128                  # 4 full groups of 128 heatmaps
NG_REM = NHM - NG_FULL * 128          # 32 in the last group
GROUPS = [128] * NG_FULL + ([NG_REM] if NG_REM else [])
NG = len(GROUPS)
ORDER = list(range(NG))               # remainder group last (small tail)
SROWS = [2, 2, 1, 2, 1]               # trailing rows on ScalarE per group
FREE = WIN * S                        # elems per heatmap window
SIGMA = 2.0 / S
DENOM = 2.0 * SIGMA * SIGMA           # 1/2048
SINV = float(np.sqrt(1.0 / DENOM))    # 45.254834
A = SINV / S
AMP = float(10.0 * np.pi / 4.0)

_cache = {}


def _build():
    import concourse.bass as bass
    import concourse.tile as tile
    from concourse import bacc, mybir
    from concourse.bass import IndirectOffsetOnAxis
    from concourse.bass_types import AP

    f32 = mybir.dt.float32
    i32 = mybir.dt.int32
    nc = bacc.Bacc("TRN2", target_bir_lowering=False, debug=False,
                   num_devices=N_CORES, dynamic_dma_scratch_size=65536)

    # host-precomputed per-heatmap tables: bias_x | bias_y | scatter offsets
    # (offsets sent as exact small-int f32 values, cast to i32 on device)
    tf = nc.dram_tensor("tf", [128, 3 * NG], f32, kind="ExternalInput")
    outs = [nc.dram_tensor(f"out{g}", [GROUPS[g] * S * S], f32,
                           kind="ExternalOutput") for g in range(NG)]
    dump = nc.dram_tensor("dump", [129 * S], f32, kind="ExternalOutput")

    o2d = [o.ap().rearrange("(a b) -> a b", b=1) for o in outs]
    d2d = dump.ap().rearrange("(a b) -> a b", b=1)

    derf = mybir.ActivationFunctionType.Derivative_Erf
    copyf = mybir.ActivationFunctionType.Copy
    op = mybir.AluOpType

    with tile.TileContext(nc) as tc:
        with tc.tile_pool(name="tabs", bufs=1) as tp, \
             tc.tile_pool(name="main", bufs=5) as mp, \
             tc.tile_pool(name="vecs", bufs=5) as vp:
            TF = tp.tile([128, 3 * NG], f32)
            nc.sync.dma_start(TF[:, :], tf.ap())

            # iotas first: they gate FX/FY and GPSIMD runs in program order
            IOTA_I = tp.tile([128, S], f32)
            nc.gpsimd.iota(IOTA_I[:], pattern=[[1, S]], base=0,
                           channel_multiplier=0,
                           allow_small_or_imprecise_dtypes=True)
            RIOTA = tp.tile([128, WIN], f32)
            nc.gpsimd.iota(RIOTA[:], pattern=[[1, WIN]], base=0,
                           channel_multiplier=0,
                           allow_small_or_imprecise_dtypes=True)

            TI = tp.tile([128, NG], i32)
            nc.gpsimd.tensor_copy(TI[:], TF[:, 2 * NG:])

            warm = tp.tile([128, 1], f32)
            nc.scalar.activation(warm[0:1, :], IOTA_I[0:1, 0:1], derf)

            # warmup scatter: zeros -> dump, absorbs SWDGE first-op cost
            # (128 partitions x 512B, distinct offsets, like the real ones)
            ZOFF = tp.tile([128, 1], i32)
            nc.gpsimd.iota(ZOFF[:], pattern=[[0, 1]], base=0,
                           channel_multiplier=S,
                           allow_small_or_imprecise_dtypes=True)
            ZSRC = tp.tile([128, S], f32)
            nc.gpsimd.memset(ZSRC[:], 0.0)
            nc.gpsimd.indirect_dma_start(
                d2d, IndirectOffsetOnAxis(ap=ZOFF[:], axis=0),
                ZSRC[:], None)

            # ---- main loop: one group of <=128 heatmaps per iteration ----
            for gi, g in enumerate(ORDER):
                n = GROUPS[g]
                drows = WIN - SROWS[g]
                FX = vp.tile([128, S], f32, tag="fx")      # fx row per hm
                nc.scalar.activation(FX[0:n, :], IOTA_I[0:n, :], derf,
                                     bias=TF[0:n, g:g + 1], scale=A)
                FY = vp.tile([128, WIN], f32, tag="fy")    # fy row per hm
                nc.scalar.activation(FY[0:n, :], RIOTA[0:n, :], derf,
                                     bias=TF[0:n, NG + g:NG + g + 1], scale=A)
                FYa = vp.tile([128, WIN], f32, tag="fya")  # AMP * fy
                if gi == 0:
                    # head: DVE is idle, shortest path to the first TT
                    nc.vector.tensor_scalar_mul(FYa[0:n, :], FY[0:n, :], AMP)
                else:
                    # steady state: keep the whole FX/FY/FYa/G-rows chain on
                    # ScalarE so no cross-engine sem hop gates the G rows
                    nc.scalar.activation(FYa[0:n, :], FY[0:n, :], copyf,
                                         scale=AMP)

                fyap = FYa[0:n, :]
                fxap = FX[0:n, :]
                G = mp.tile([128, FREE], f32, tag="g")
                in0 = AP(tensor=fyap.tensor, offset=fyap.offset,
                         ap=[[fyap.ap[0][0], n], [1, drows], [0, S]])
                in1 = AP(tensor=fxap.tensor, offset=fxap.offset,
                         ap=[[fxap.ap[0][0], n], [0, drows], [1, S]])
                nc.vector.tensor_tensor(G[0:n, 0:drows * S], in0, in1, op.mult)
                for r in range(drows, WIN):
                    nc.scalar.activation(G[0:n, r * S:(r + 1) * S],
                                         FX[0:n, :], copyf,
                                         scale=FYa[0:n, r:r + 1])
                nc.gpsimd.indirect_dma_start(
                    o2d[g],
                    IndirectOffsetOnAxis(ap=TI[0:n, g:g + 1], axis=0),
                    G[0:n, :], None)

    nc.compile()
    return nc


def _tables(coords_core):
    """Host-side per-heatmap tables for one core: tf [128, 3*NG] f32."""
    c = np.ascontiguousarray(coords_core, dtype=np.float32).reshape(-1, 2)
    x = c[:, 0].astype(np.float64)
    y = c[:, 1].astype(np.float64)
    # center window on the true peak row (pixel center grid): c = S*y - 0.5
    jo = np.clip(np.rint(S * y - 0.5 - (WIN - 1) / 2).astype(np.int64),
                 0, S - WIN)
    p = np.arange(NHM, dtype=np.int64) % 128
    bx = (A * 0.5 - SINV * x).astype(np.float32)
    by = (A * jo + A * 0.5 - SINV * y).astype(np.float32)
    off = (p * (S * S) + jo * S).astype(np.float32)  # exact ints < 2^24

    npad = NG * 128

    def pack(v):
        q = np.zeros(npad, dtype=np.float32)
        q[:NHM] = v
        return np.ascontiguousarray(q.reshape(NG, 128).T)

    return np.concatenate([pack(bx), pack(by), pack(off)], axis=1)


def _get_nc():
    if "nc" not in _cache:
        _cache["nc"] = _build()
    return _cache["nc"]


def _run(coords_full, trace=False):
    from concourse.bass_utils import run_bass_kernel_spmd

    coords_full = np.ascontiguousarray(np.asarray(coords_full, dtype=np.float32))
    assert coords_full.shape == (B_TOTAL, 2 * NUM_CLASS)
    nc = _get_nc()
    in_maps = [{"tf": _tables(coords_full[i * B_LOC:(i + 1) * B_LOC])}
               for i in range(N_CORES)]
    br = run_bass_kernel_spmd(nc, in_maps, core_ids=list(range(N_CORES)),
                              trace=trace)
    parts = []
    for i in range(N_CORES):
        r = br.results[i]
        core = np.concatenate([r[f"out{g}"] for g in range(NG)])
        parts.append(core.reshape(B_LOC, NUM_CLASS, S, S))
    full = np.concatenate(parts, axis=0)
    return full, br


def kernel(coords):
    return _run(coords, trace=False)[0]


# revision 25
# speedup vs baseline: 1.0016x; 1.0016x over previous
"""Coord2HeatmapNet Trainium2 kernel.

out[b,c,j,i] = 10*exp(-(((i+.5)/128 - x)^2 + ((j+.5)/128 - y)^2) / (2*(2/128)^2))

Exploited structure:
  * Separable: each heatmap = fy[j] (x) fx[i] outer product.
  * The gaussian decays fast: a WIN-row window per heatmap carries everything
    above ~2.2e-2 absolute (tolerance is 0.2 abs); the pre-zeroed output
    buffers keep the rest at 0.
  * Derivative_Erf activation = 2/sqrt(pi)*exp(-t^2): one ScalarE op per
    gaussian factor vector.
  * Per-heatmap scalar tables (activation biases, scatter offsets) are tiny
    index arithmetic -> precomputed on host, DMAed in as one [128,_] table.
  * Layout: one heatmap per PARTITION. Partition p of group g holds the whole
    WINx128 window of heatmap k=g*128+p as WIN*128 contiguous floats. The
    outer product is split between DVE (leading rows, one broadcast
    tensor_tensor) and ScalarE (trailing SROWS rows, activation-Copy with
    per-partition scale); the write-out is ONE indirect scatter DMA per group
    (one offset per partition, WIN*512B contiguous per heatmap at its
    data-dependent window position).
  * Each group scatters into its OWN ExternalOutput dram tensor -> no
    write-after-write hazard between groups, so the scatters overlap on the
    16 SDMA engines instead of serializing on completion semaphores.
  * A dummy warmup scatter (to a throwaway output tensor) right after pool
    setup absorbs the ~1us first-indirect-DMA ucode warmup off the critical
    path.

Sharding: pure data parallel, 8 batches per core across 8 NeuronCores.
"""
import sys

for _p in ("/opt/trn_rl_repo", "/root/.axon_site", "/root/.axon_site/_ro/trn_rl_repo",
           "/root/.axon_site/_ro/pypackages"):
    if _p not in sys.path:
        sys.path.append(_p)

import numpy as np

S = 128
NUM_CLASS = 68
B_TOTAL = 64
N_CORES = 8
B_LOC = B_TOTAL // N_CORES            # 8 batches per core
NHM = B_LOC * NUM_CLASS               # 544 heatmaps per core
WIN = 12                              # window rows per heatmap
NG_FULL = NHM // 128                  # 4 full groups of 128 heatmaps
NG_REM = NHM - NG_FULL * 128          # 32 in the last group
GROUPS = [128] * NG_FULL + ([NG_REM] if NG_REM else [])
NG = len(GROUPS)
ORDER = list(range(NG))               # remainder group last (small tail)
SROWS = [2, 2, 1, 2, 1]               # trailing rows on ScalarE per group
FREE = WIN * S                        # elems per heatmap window
SIGMA = 2.0 / S
DENOM = 2.0 * SIGMA * SIGMA           # 1/2048
SINV = float(np.sqrt(1.0 / DENOM))    # 45.254834
A = SINV / S
AMP = float(10.0 * np.pi / 4.0)

_cache = {}


def _build():
    import concourse.bass as bass
    import concourse.tile as tile
    from concourse import bacc, mybir
    from concourse.bass import IndirectOffsetOnAxis
    from concourse.bass_types import AP

    f32 = mybir.dt.float32
    i32 = mybir.dt.int32
    nc = bacc.Bacc("TRN2", target_bir_lowering=False, debug=False,
                   num_devices=N_CORES, dynamic_dma_scratch_size=65536)

    # host-precomputed per-heatmap tables: bias_x | bias_y | scatter offsets
    # (offsets sent as exact small-int f32 values, cast to i32 on device)
    tf = nc.dram_tensor("tf", [128, 3 * NG], f32, kind="ExternalInput")
    outs = [nc.dram_tensor(f"out{g}", [GROUPS[g] * S * S], f32,
                           kind="ExternalOutput") for g in range(NG)]
    dump = nc.dram_tensor("dump", [129 * S], f32, kind="ExternalOutput")

    o2d = [o.ap().rearrange("(a b) -> a b", b=1) for o in outs]
    d2d = dump.ap().rearrange("(a b) -> a b", b=1)

    derf = mybir.ActivationFunctionType.Derivative_Erf
    copyf = mybir.ActivationFunctionType.Copy
    op = mybir.AluOpType

    with tile.TileContext(nc) as tc:
        with tc.tile_pool(name="tabs", bufs=1) as tp, \
             tc.tile_pool(name="main", bufs=5) as mp, \
             tc.tile_pool(name="vecs", bufs=3) as vp:
            TF = tp.tile([128, 3 * NG], f32)
            nc.sync.dma_start(TF[:, :], tf.ap())

            # iotas first: they gate FX/FY and GPSIMD runs in program order
            IOTA_I = tp.tile([128, S], f32)
            nc.gpsimd.iota(IOTA_I[:], pattern=[[1, S]], base=0,
                           channel_multiplier=0,
                           allow_small_or_imprecise_dtypes=True)
            RIOTA = tp.tile([128, WIN], f32)
            nc.gpsimd.iota(RIOTA[:], pattern=[[1, WIN]], base=0,
                           channel_multiplier=0,
                           allow_small_or_imprecise_dtypes=True)

            TI = tp.tile([128, NG], i32)
            nc.gpsimd.tensor_copy(TI[:], TF[:, 2 * NG:])

            warm = tp.tile([128, 1], f32)
            nc.scalar.activation(warm[0:1, :], IOTA_I[0:1, 0:1], derf)

            # warmup scatter: zeros -> dump, absorbs SWDGE first-op cost
            # (128 partitions x 512B, distinct offsets, like the real ones)
            ZOFF = tp.tile([128, 1], i32)
            nc.gpsimd.iota(ZOFF[:], pattern=[[0, 1]], base=0,
                           channel_multiplier=S,
                           allow_small_or_imprecise_dtypes=True)
            ZSRC = tp.tile([128, S], f32)
            nc.gpsimd.memset(ZSRC[:], 0.0)
            nc.gpsimd.indirect_dma_start(
                d2d, IndirectOffsetOnAxis(ap=ZOFF[:], axis=0),
                ZSRC[:], None)

            # ---- main loop: one group of <=128 heatmaps per iteration ----
            for gi, g in enumerate(ORDER):
                n = GROUPS[g]
                drows = WIN - SROWS[g]
                FX = vp.tile([128, S], f32, tag="fx")      # fx row per hm
                nc.scalar.activation(FX[0:n, :], IOTA_I[0:n, :], derf,
                                     bias=TF[0:n, g:g + 1], scale=A)
                FY = vp.tile([128, WIN], f32, tag="fy")    # fy row per hm
                nc.scalar.activation(FY[0:n, :], RIOTA[0:n, :], derf,
                                     bias=TF[0:n, NG + g:NG + g + 1], scale=A)
                FYa = vp.tile([128, WIN], f32, tag="fya")  # AMP * fy
                if gi == 0:
                    # head: DVE is idle, shortest path to the first TT
                    nc.vector.tensor_scalar_mul(FYa[0:n, :], FY[0:n, :], AMP)
                else:
                    # steady state: keep the whole FX/FY/FYa/G-rows chain on
                    # ScalarE so no cross-engine sem hop gates the G rows
                    nc.scalar.activation(FYa[0:n, :], FY[0:n, :], copyf,
                                         scale=AMP)

                fyap = FYa[0:n, :]
                fxap = FX[0:n, :]
                G = mp.tile([128, FREE], f32, tag="g")
                in0 = AP(tensor=fyap.tensor, offset=fyap.offset,
                         ap=[[fyap.ap[0][0], n], [1, drows], [0, S]])
                in1 = AP(tensor=fxap.tensor, offset=fxap.offset,
                         ap=[[fxap.ap[0][0], n], [0, drows], [1, S]])
                nc.vector.tensor_tensor(G[0:n, 0:drows * S], in0, in1, op.mult)
                for r in range(drows, WIN):
                    nc.scalar.activation(G[0:n, r * S:(r + 1) * S],
                                         FX[0:n, :], copyf,
                                         scale=FYa[0:n, r:r + 1])
                nc.gpsimd.indirect_dma_start(
                    o2d[g],
                    IndirectOffsetOnAxis(ap=TI[0:n, g:g + 1], axis=0),
                    G[0:n, :], None)

    nc.compile()
    return nc


def _tables(coords_core):
    """Host-side per-heatmap tables for one core: tf [128, 3*NG] f32."""
    c = np.ascontiguousarray(coords_core, dtype=np.float32).reshape(-1, 2)
    x = c[:, 0].astype(np.float64)
    y = c[:, 1].astype(np.float64)
    # center window on the true peak row (pixel center grid): c = S*y - 0.5
    jo = np.clip(np.rint(S * y - 0.5 - (WIN - 1) / 2).astype(np.int64),
                 0, S - WIN)
    p = np.arange(NHM, dtype=np.int64) % 128
    bx = (A * 0.5 - SINV * x).astype(np.float32)
    by = (A * jo + A * 0.5 - SINV * y).astype(np.float32)
    off = (p * (S * S) + jo * S).astype(np.float32)  # exact ints < 2^24

    npad = NG * 128

    def pack(v):
        q = np.zeros(npad, dtype=np.float32)
        q[:NHM] = v
        return np.ascontiguousarray(q.reshape(NG, 128).T)

    return np.concatenate([pack(bx), pack(by), pack(off)], axis=1)


def _get_nc():
    if "nc" not in _cache:
        _cache["nc"] = _build()
    return _cache["nc"]


def _run(coords_full, trace=False):
    from concourse.bass_utils import run_bass_kernel_spmd

    coords_full = np.ascontiguousarray(np.asarray(coords_full, dtype=np.float32))
    assert coords_full.shape == (B_TOTAL, 2 * NUM_CLASS)
    nc = _get_nc()
    in_maps = [{"tf": _tables(coords_full[i * B_LOC:(i + 1) * B_LOC])}
               for i in range(N_CORES)]
    br = run_bass_kernel_spmd(nc, in_maps, core_ids=list(range(N_CORES)),
                              trace=trace)
    parts = []
    for i in range(N_CORES):
        r = br.results[i]
        core = np.concatenate([r[f"out{g}"] for g in range(NG)])
        parts.append(core.reshape(B_LOC, NUM_CLASS, S, S))
    full = np.concatenate(parts, axis=0)
    return full, br


def kernel(coords):
    return _run(coords, trace=False)[0]
